# revision 8
# baseline (speedup 1.0000x reference)
"""MoE routing kernel (2 experts, D=128 -> H=512 -> O=2) for 8 Trainium2 cores.

Strategy: pure data parallel. x is sharded along batch across 8 cores; the
tiny expert weights are replicated (pre-packed host-side into PE-friendly
layouts). Per 512-sample block on each core:

  1. DMA x block (natural [128b, 4s, 128d] tiles) HBM->SBUF
  2. PE transposes the 4 sub-tiles -> xT [128d, 512b] (PSUM), ACT copies to
     SBUF (rounded to fp32r)
  3. PE layer-1: 8 fp32r matmuls (w1 tiles stationary, xT moving) -> z PSUM
  4. ACT/DVE: fused relu(z + b1) (per-partition bias) -> h SBUF fp32r
  5. PE layer-2 (streaming): 8 fp32r matmuls, w2 [128,4] stationary, h moving
     -> out_all [4(2e+o), 512b] PSUM
  6. DVE: routing dot q = x . (p1-p0) from the natural tiles (accum_out)
  7. PE: transpose out_all back to [128b, 4] (+rank-1 matmul adds b2),
     DVE selects the routed expert, DMA out
"""

import numpy as np

import concourse.bacc as bacc
import concourse.bass as bass
import concourse.mybir as mybir
import concourse.tile as tile
from concourse.bass_utils import run_bass_kernel_spmd

F32 = mybir.dt.float32
F32R = mybir.dt.float32r

N_CORES = 8
D = 128
H = 512
E = 2
O = 2
NJ = (E * H) // 128  # 8 hidden k-tiles of 128
BLK = 512            # samples per block
NSUB = BLK // 128    # 4 sub-tiles per block


def _build_program(n_shard: int):
    """Build the per-core Bass program for a shard of n_shard samples."""
    nblk = n_shard // BLK
    assert nblk * BLK == n_shard

    nc = bacc.Bacc(
        "TRN2",
        target_bir_lowering=False,
        debug=False,
        enable_asserts=False,
        num_devices=1,
    )

    x = nc.dram_tensor("x", [n_shard, D], F32, kind="ExternalInput").ap()
    w1t = nc.dram_tensor("w1t", [128, E * H], F32R, kind="ExternalInput").ap()
    w2r = nc.dram_tensor("w2r", [128, NJ, E * O], F32R, kind="ExternalInput").ap()
    b1c = nc.dram_tensor("b1c", [128, NJ], F32, kind="ExternalInput").ap()
    rvecb = nc.dram_tensor("rvecb", [128, D], F32, kind="ExternalInput").ap()
    b2bc = nc.dram_tensor("b2bc", [128, NSUB, E * O], F32, kind="ExternalInput").ap()
    ident = nc.dram_tensor("ident", [128, 128], F32, kind="ExternalInput").ap()
    thrv = nc.dram_tensor("thrv", [128, 1], F32, kind="ExternalInput").ap()
    out = nc.dram_tensor("out", [n_shard, O], F32, kind="ExternalOutput").ap()

    with tile.TileContext(nc) as tc:
        _body(tc, nblk, x, w1t, w2r, b1c, rvecb, b2bc, ident, thrv, out)

    nc.compile()
    return nc


def _body(tc, nblk, x, w1t, w2r, b1c, rvecb, b2bc, ident, thrv, out):
    nc = tc.nc
    Relu = mybir.ActivationFunctionType.Relu
    Alu = mybir.AluOpType

    with (
        tc.tile_pool(name="consts", bufs=1) as cpool,
        tc.tile_pool(name="xn", bufs=4) as xn_pool,
        tc.tile_pool(name="xt", bufs=3) as xt_pool,
        tc.tile_pool(name="h", bufs=3) as h_pool,
        tc.tile_pool(name="small", bufs=4) as s_pool,
        tc.tile_pool(name="xtp", bufs=2, space="PSUM") as xtp_pool,
        tc.tile_pool(name="zp", bufs=3, space="PSUM") as zp_pool,
        tc.tile_pool(name="op", bufs=2, space="PSUM") as op_pool,
        tc.tile_pool(name="ot", bufs=1, space="PSUM") as ot_pool,
    ):
        # --- load constants once ---
        w1t_sb = cpool.tile([128, E * H], F32R)
        nc.sync.dma_start(w1t_sb[:], w1t)
        w2r_sb = cpool.tile([128, NJ, E * O], F32R)
        nc.sync.dma_start(w2r_sb[:], w2r)
        b1c_sb = cpool.tile([128, NJ], F32)
        nc.sync.dma_start(b1c_sb[:], b1c)
        rvecb_sb = cpool.tile([128, D], F32)
        nc.sync.dma_start(rvecb_sb[:], rvecb)
        b2bc_sb = cpool.tile([128, NSUB, E * O], F32)
        nc.sync.dma_start(b2bc_sb[:], b2bc)
        id_sb = cpool.tile([128, 128], F32)
        nc.sync.dma_start(id_sb[:], ident)
        thr_sb = cpool.tile([128, 1], F32)
        nc.sync.dma_start(thr_sb[:], thrv)

        xv = x.rearrange("(n s p) d -> n p s d", p=128, s=NSUB)
        ov = out.rearrange("(n s p) o -> n p s o", p=128, s=NSUB)

        for bi in range(nblk):
            # 1. load natural x tiles [128b, 4s, 128d]
            xn = xn_pool.tile([128, NSUB, D], F32)
            nc.sync.dma_start(xn[:], xv[bi])

            # 2. transpose -> xT [128d, (s p)b]
            xtp = xtp_pool.tile([128, NSUB, 128], F32)
            for s in range(NSUB):
                nc.tensor.transpose(xtp[:, s, :], xn[:, s, :], id_sb[:])
            xt_sb = xt_pool.tile([128, BLK], F32R)
            nc.scalar.copy(xt_sb[:], xtp.rearrange("p s b -> p (s b)"))

            # 6. routing dot: q[b] = x[b] . rvec  (DVE, fp32)
            q_sb = s_pool.tile([128, NSUB], F32, tag="q")
            for s in range(NSUB):
                scr = s_pool.tile([128, D], F32, tag="scr")
                nc.vector.scalar_tensor_tensor(
                    out=scr[:],
                    in0=xn[:, s, :],
                    scalar=0.0,
                    in1=rvecb_sb[:],
                    op0=Alu.bypass,
                    op1=Alu.mult,
                    accum_out=q_sb[:, s : s + 1],
                )

            # 3. layer 1: z_j = w1_j^T @ xT   (fp32r)
            h = h_pool.tile([128, NJ, BLK], F32R)
            for j in range(NJ):
                zp = zp_pool.tile([128, BLK], F32)
                nc.tensor.matmul(
                    zp[:],
                    lhsT=w1t_sb[:, j * 128 : (j + 1) * 128],
                    rhs=xt_sb[:],
                    start=True,
                    stop=True,
                )
                # 4. relu(z + b1) -> h (fp32r), alternate ACT/DVE
                if j not in (1, 4, 6):
                    nc.scalar.activation(
                        h[:, j, :], zp[:], Relu, bias=b1c_sb[:, j : j + 1], scale=1.0
                    )
                else:
                    nc.vector.tensor_scalar(
                        out=h[:, j, :],
                        in0=zp[:],
                        scalar1=b1c_sb[:, j : j + 1],
                        scalar2=0.0,
                        op0=Alu.add,
                        op1=Alu.max,
                    )

            # 5. layer 2 streaming: out_all [4(2e+o), 512b]
            op_ps = op_pool.tile([4, BLK], F32)
            for j in range(NJ):
                nc.tensor.matmul(
                    op_ps[:],
                    lhsT=w2r_sb[:, j, :],
                    rhs=h[:, j, :],
                    start=(j == 0),
                    stop=(j == NJ - 1),
                )
            oall_sb = s_pool.tile([4, BLK], F32, tag="oall")
            nc.vector.tensor_copy(oall_sb[:], op_ps[:])

            # 7. transpose out_all to [128b, 4] + add b2 (rank-1 accumulate)
            ot_ps = ot_pool.tile([128, NSUB, E * O], F32)
            for s in range(NSUB):
                nc.tensor.matmul(
                    ot_ps[:, s, :],
                    lhsT=oall_sb[:, s * 128 : (s + 1) * 128],
                    rhs=id_sb[0:4, 0:4],
                    is_transpose=True,
                    start=True,
                    stop=True,
                )

            # select expert by routing mask, write out
            osb = s_pool.tile([128, NSUB, E * O], F32, tag="osb")
            nc.vector.tensor_tensor(osb[:], ot_ps[:], b2bc_sb[:], Alu.add)
            m_sb = s_pool.tile([128, NSUB], F32, tag="m")
            # expert0 wins ties: q <= thr -> 1.0
            nc.vector.tensor_scalar(
                out=m_sb[:],
                in0=q_sb[:],
                scalar1=thr_sb[:],
                scalar2=None,
                op0=Alu.is_le,
            )
            m2 = s_pool.tile([128, NSUB, O], F32, tag="m2")
            nc.vector.tensor_copy(m2[:], m_sb[:].broadcast_to([128, NSUB, O]))
            d_sb = s_pool.tile([128, NSUB, O], F32, tag="d")
            nc.vector.tensor_tensor(
                d_sb[:], osb[:, :, 0:O], osb[:, :, O : 2 * O], Alu.subtract
            )
            nc.vector.tensor_tensor(d_sb[:], d_sb[:], m2[:], Alu.mult)
            osel = s_pool.tile([128, NSUB, O], F32, tag="osel")
            nc.vector.tensor_tensor(
                osel[:], d_sb[:], osb[:, :, O : 2 * O], Alu.add
            )
            nc.sync.dma_start(ov[bi], osel[:])


def _pack_consts(w1, b1, w2, b2, prototypes):
    w1 = np.asarray(w1, np.float32)
    b1 = np.asarray(b1, np.float32)
    w2 = np.asarray(w2, np.float32)
    b2 = np.asarray(b2, np.float32)
    p = np.asarray(prototypes, np.float64)

    w1t = np.ascontiguousarray(np.transpose(w1, (2, 0, 1)).reshape(D, E * H))
    w2r = np.zeros((128, NJ, E * O), np.float32)
    b1c = np.zeros((128, NJ), np.float32)
    for e in range(E):
        for k in range(H // 128):
            j = e * (H // 128) + k
            for o in range(O):
                w2r[:, j, 2 * e + o] = w2[e, o, k * 128 : (k + 1) * 128]
            b1c[:, j] = b1[e, k * 128 : (k + 1) * 128]
    rvec = (p[1] - p[0]).astype(np.float32)
    rvecb = np.tile(rvec[None, :], (128, 1))
    thr = np.float32((p[1] @ p[1] - p[0] @ p[0]) / 2.0)
    thrv = np.full((128, 1), thr, np.float32)
    b2r = np.zeros((E * O,), np.float32)
    for e in range(E):
        for o in range(O):
            b2r[2 * e + o] = b2[e, o]
    b2bc = np.tile(b2r[None, None, :], (128, NSUB, 1))
    ident = np.eye(128, dtype=np.float32)
    return dict(
        w1t=w1t, w2r=w2r, b1c=b1c, rvecb=rvecb, b2bc=b2bc,
        ident=ident, thrv=thrv,
    )


_PROG_CACHE = {}


def _get_program(n_shard):
    if n_shard not in _PROG_CACHE:
        _PROG_CACHE[n_shard] = _build_program(n_shard)
    return _PROG_CACHE[n_shard]


def kernel(x, w1, b1, w2, b2, prototypes, _trace=False):
    x = np.ascontiguousarray(np.asarray(x, np.float32))
    btot = x.shape[0]
    n_shard = btot // N_CORES
    nc = _get_program(n_shard)
    consts = _pack_consts(w1, b1, w2, b2, prototypes)

    in_maps = []
    for c in range(N_CORES):
        m = dict(consts)
        m["x"] = x[c * n_shard : (c + 1) * n_shard]
        in_maps.append(m)

    res = run_bass_kernel_spmd(
        nc, in_maps, core_ids=list(range(N_CORES)), trace=_trace
    )
    outs = [res.results[c]["out"] for c in range(N_CORES)]
    full = np.concatenate(outs, axis=0)
    if _trace:
        return full, res
    return full


# revision 9
# speedup vs baseline: 1.1859x; 1.1859x over previous
"""MoE routing kernel (2 experts, D=128 -> H=512 -> O=2) for 8 Trainium2 cores.

Strategy: pure data parallel. x is sharded along batch across 8 cores; the
tiny expert weights are replicated (pre-packed host-side into PE-friendly
layouts). Per 512-sample block on each core:

  1. DMA x block (natural [128b, 4s, 128d] tiles) HBM->SBUF
  2. PE transposes the 4 sub-tiles -> xT [128d, 512b] (PSUM), ACT copies to
     SBUF (rounded to fp32r)
  3. PE layer-1: 8 fp32r matmuls (w1 tiles stationary, xT moving) -> z PSUM
  4. ACT/DVE: fused relu(z + b1) (per-partition bias) -> h SBUF fp32r
  5. PE layer-2 (streaming): 8 fp32r matmuls, w2 [128,4] stationary, h moving
     -> out_all [4(2e+o), 512b] PSUM
  6. DVE: routing dot q = x . (p1-p0) from the natural tiles (accum_out)
  7. PE: transpose out_all back to [128b, 4] (+rank-1 matmul adds b2),
     DVE selects the routed expert, DMA out
"""

import numpy as np

import concourse.bacc as bacc
import concourse.bass as bass
import concourse.mybir as mybir
import concourse.tile as tile
from concourse.bass_utils import run_bass_kernel_spmd

F32 = mybir.dt.float32
F32R = mybir.dt.float32r

N_CORES = 8
D = 128
H = 512
E = 2
O = 2
NJ = (E * H) // 128  # 8 hidden k-tiles of 128
BLK = 512            # samples per block
NSUB = BLK // 128    # 4 sub-tiles per block


def _build_program(n_shard: int):
    """Build the per-core Bass program for a shard of n_shard samples."""
    nblk = n_shard // BLK
    assert nblk * BLK == n_shard

    nc = bacc.Bacc(
        "TRN2",
        target_bir_lowering=False,
        debug=False,
        enable_asserts=False,
        num_devices=1,
    )

    x = nc.dram_tensor("x", [n_shard, D], F32, kind="ExternalInput").ap()
    w1t = nc.dram_tensor("w1t", [128, E * H], F32R, kind="ExternalInput").ap()
    w2r = nc.dram_tensor("w2r", [128, NJ, E * O], F32R, kind="ExternalInput").ap()
    b1c = nc.dram_tensor("b1c", [128, NJ], F32, kind="ExternalInput").ap()
    rvecb = nc.dram_tensor("rvecb", [128, D], F32, kind="ExternalInput").ap()
    b2bc = nc.dram_tensor("b2bc", [128, NSUB, E * O], F32, kind="ExternalInput").ap()
    ident = nc.dram_tensor("ident", [128, 128], F32, kind="ExternalInput").ap()
    thrv = nc.dram_tensor("thrv", [128, 1], F32, kind="ExternalInput").ap()
    out = nc.dram_tensor("out", [n_shard, O], F32, kind="ExternalOutput").ap()

    with tile.TileContext(nc) as tc:
        _body(tc, nblk, x, w1t, w2r, b1c, rvecb, b2bc, ident, thrv, out)

    nc.compile()
    return nc


def _body(tc, nblk, x, w1t, w2r, b1c, rvecb, b2bc, ident, thrv, out):
    nc = tc.nc
    Relu = mybir.ActivationFunctionType.Relu
    Alu = mybir.AluOpType

    with (
        tc.tile_pool(name="consts", bufs=1) as cpool,
        tc.tile_pool(name="xn", bufs=3) as xn_pool,
        tc.tile_pool(name="xt", bufs=2) as xt_pool,
        tc.tile_pool(name="h", bufs=2) as h_pool,
        tc.tile_pool(name="small", bufs=3) as s_pool,
        tc.tile_pool(name="xtp", bufs=2, space="PSUM") as xtp_pool,
        tc.tile_pool(name="zp", bufs=3, space="PSUM") as zp_pool,
        tc.tile_pool(name="op", bufs=2, space="PSUM") as op_pool,
        tc.tile_pool(name="ot", bufs=1, space="PSUM") as ot_pool,
    ):
        # --- load constants once ---
        w1t_sb = cpool.tile([128, E * H], F32R)
        nc.sync.dma_start(w1t_sb[:], w1t)
        w2r_sb = cpool.tile([128, NJ, E * O], F32R)
        nc.sync.dma_start(w2r_sb[:], w2r)
        b1c_sb = cpool.tile([128, NJ], F32)
        nc.sync.dma_start(b1c_sb[:], b1c)
        rvecb_sb = cpool.tile([128, D], F32)
        nc.sync.dma_start(rvecb_sb[:], rvecb)
        b2bc_sb = cpool.tile([128, NSUB, E * O], F32)
        nc.sync.dma_start(b2bc_sb[:], b2bc)
        id_sb = cpool.tile([128, 128], F32)
        nc.sync.dma_start(id_sb[:], ident)
        thr_sb = cpool.tile([128, 1], F32)
        nc.sync.dma_start(thr_sb[:], thrv)

        xv = x.rearrange("(n s p) d -> n p s d", p=128, s=NSUB)
        ov = out.rearrange("(n s p) o -> n p s o", p=128, s=NSUB)

        for bi in range(nblk):
            # 1. load natural x tiles [128b, 4s, 128d]
            xn = xn_pool.tile([128, NSUB, D], F32)
            nc.sync.dma_start(xn[:], xv[bi])

            # 2. transpose -> xT [128d, (s p)b]
            xtp = xtp_pool.tile([128, NSUB, 128], F32)
            for s in range(NSUB):
                nc.tensor.transpose(xtp[:, s, :], xn[:, s, :], id_sb[:])
            xt_sb = xt_pool.tile([128, BLK], F32R)
            nc.scalar.copy(xt_sb[:], xtp.rearrange("p s b -> p (s b)"))

            # 6. routing dot: q[b] = x[b] . rvec  (DVE, fp32)
            q_sb = s_pool.tile([128, NSUB], F32, tag="q")
            for s in range(NSUB):
                scr = s_pool.tile([128, D], F32, tag="scr")
                nc.vector.scalar_tensor_tensor(
                    out=scr[:],
                    in0=xn[:, s, :],
                    scalar=0.0,
                    in1=rvecb_sb[:],
                    op0=Alu.bypass,
                    op1=Alu.mult,
                    accum_out=q_sb[:, s : s + 1],
                )

            # 3. layer 1: z_j = w1_j^T @ xT   (fp32r)
            h = h_pool.tile([128, NJ, BLK], F32R)
            for j in range(NJ):
                zp = zp_pool.tile([128, BLK], F32)
                nc.tensor.matmul(
                    zp[:],
                    lhsT=w1t_sb[:, j * 128 : (j + 1) * 128],
                    rhs=xt_sb[:],
                    start=True,
                    stop=True,
                )
                # 4. relu(z + b1) -> h (fp32r), alternate ACT/DVE
                if j % 2 == 0:
                    nc.scalar.activation(
                        h[:, j, :], zp[:], Relu, bias=b1c_sb[:, j : j + 1], scale=1.0
                    )
                else:
                    nc.vector.tensor_scalar(
                        out=h[:, j, :],
                        in0=zp[:],
                        scalar1=b1c_sb[:, j : j + 1],
                        scalar2=0.0,
                        op0=Alu.add,
                        op1=Alu.max,
                    )

            # 5. layer 2 streaming: out_all [4(2e+o), 512b]
            op_ps = op_pool.tile([4, BLK], F32)
            for j in range(NJ):
                nc.tensor.matmul(
                    op_ps[:],
                    lhsT=w2r_sb[:, j, :],
                    rhs=h[:, j, :],
                    start=(j == 0),
                    stop=(j == NJ - 1),
                )
            oall_sb = s_pool.tile([4, BLK], F32, tag="oall")
            nc.scalar.copy(oall_sb[:], op_ps[:])

            # 7. transpose out_all to [128b, 4] + add b2 (rank-1 accumulate)
            ot_ps = ot_pool.tile([128, NSUB, E * O], F32)
            for s in range(NSUB):
                nc.tensor.matmul(
                    ot_ps[:, s, :],
                    lhsT=oall_sb[:, s * 128 : (s + 1) * 128],
                    rhs=id_sb[0:4, 0:4],
                    is_transpose=True,
                    start=True,
                    stop=True,
                )

            # select expert by routing mask, write out
            osb = s_pool.tile([128, NSUB, E * O], F32, tag="osb")
            nc.vector.tensor_tensor(osb[:], ot_ps[:], b2bc_sb[:], Alu.add)
            m_sb = s_pool.tile([128, NSUB], F32, tag="m")
            # expert0 wins ties: q <= thr -> 1.0
            nc.vector.tensor_scalar(
                out=m_sb[:],
                in0=q_sb[:],
                scalar1=thr_sb[:],
                scalar2=None,
                op0=Alu.is_le,
            )
            m2 = s_pool.tile([128, NSUB, O], F32, tag="m2")
            nc.vector.tensor_copy(m2[:], m_sb[:].broadcast_to([128, NSUB, O]))
            d_sb = s_pool.tile([128, NSUB, O], F32, tag="d")
            nc.vector.tensor_tensor(
                d_sb[:], osb[:, :, 0:O], osb[:, :, O : 2 * O], Alu.subtract
            )
            nc.vector.tensor_tensor(d_sb[:], d_sb[:], m2[:], Alu.mult)
            osel = s_pool.tile([128, NSUB, O], F32, tag="osel")
            nc.vector.tensor_tensor(
                osel[:], d_sb[:], osb[:, :, O : 2 * O], Alu.add
            )
            nc.sync.dma_start(ov[bi], osel[:])


def _pack_consts(w1, b1, w2, b2, prototypes):
    w1 = np.asarray(w1, np.float32)
    b1 = np.asarray(b1, np.float32)
    w2 = np.asarray(w2, np.float32)
    b2 = np.asarray(b2, np.float32)
    p = np.asarray(prototypes, np.float64)

    w1t = np.ascontiguousarray(np.transpose(w1, (2, 0, 1)).reshape(D, E * H))
    w2r = np.zeros((128, NJ, E * O), np.float32)
    b1c = np.zeros((128, NJ), np.float32)
    for e in range(E):
        for k in range(H // 128):
            j = e * (H // 128) + k
            for o in range(O):
                w2r[:, j, 2 * e + o] = w2[e, o, k * 128 : (k + 1) * 128]
            b1c[:, j] = b1[e, k * 128 : (k + 1) * 128]
    rvec = (p[1] - p[0]).astype(np.float32)
    rvecb = np.tile(rvec[None, :], (128, 1))
    thr = np.float32((p[1] @ p[1] - p[0] @ p[0]) / 2.0)
    thrv = np.full((128, 1), thr, np.float32)
    b2r = np.zeros((E * O,), np.float32)
    for e in range(E):
        for o in range(O):
            b2r[2 * e + o] = b2[e, o]
    b2bc = np.tile(b2r[None, None, :], (128, NSUB, 1))
    ident = np.eye(128, dtype=np.float32)
    return dict(
        w1t=w1t, w2r=w2r, b1c=b1c, rvecb=rvecb, b2bc=b2bc,
        ident=ident, thrv=thrv,
    )


_PROG_CACHE = {}


def _get_program(n_shard):
    if n_shard not in _PROG_CACHE:
        _PROG_CACHE[n_shard] = _build_program(n_shard)
    return _PROG_CACHE[n_shard]


def kernel(x, w1, b1, w2, b2, prototypes, _trace=False):
    x = np.ascontiguousarray(np.asarray(x, np.float32))
    btot = x.shape[0]
    n_shard = btot // N_CORES
    nc = _get_program(n_shard)
    consts = _pack_consts(w1, b1, w2, b2, prototypes)

    in_maps = []
    for c in range(N_CORES):
        m = dict(consts)
        m["x"] = x[c * n_shard : (c + 1) * n_shard]
        in_maps.append(m)

    res = run_bass_kernel_spmd(
        nc, in_maps, core_ids=list(range(N_CORES)), trace=_trace
    )
    outs = [res.results[c]["out"] for c in range(N_CORES)]
    full = np.concatenate(outs, axis=0)
    if _trace:
        return full, res
    return full


# revision 10
# speedup vs baseline: 1.2082x; 1.0188x over previous
"""MoE routing kernel (2 experts, D=128 -> H=512 -> O=2) for 8 Trainium2 cores.

Strategy: pure data parallel. x is sharded along batch across 8 cores; the
tiny expert weights are replicated (pre-packed host-side into PE-friendly
layouts). Per 512-sample block on each core:

  1. DMA x block (natural [128b, 4s, 128d] tiles) HBM->SBUF
  2. PE transposes the 4 sub-tiles -> xT [128d, 512b] (PSUM), ACT copies to
     SBUF (rounded to fp32r)
  3. PE layer-1: 8 fp32r matmuls (w1 tiles stationary, xT moving) -> z PSUM
  4. ACT/DVE: fused relu(z + b1) (per-partition bias) -> h SBUF fp32r
  5. PE layer-2 (streaming): 8 fp32r matmuls, w2 [128,4] stationary, h moving
     -> out_all [4(2e+o), 512b] PSUM
  6. DVE: routing dot q = x . (p1-p0) from the natural tiles (accum_out)
  7. PE: transpose out_all back to [128b, 4] (+rank-1 matmul adds b2),
     DVE selects the routed expert, DMA out
"""

import numpy as np

import concourse.bacc as bacc
import concourse.bass as bass
import concourse.mybir as mybir
import concourse.tile as tile
from concourse.bass_utils import run_bass_kernel_spmd

F32 = mybir.dt.float32
F32R = mybir.dt.float32r

N_CORES = 8
D = 128
H = 512
E = 2
O = 2
NJ = (E * H) // 128  # 8 hidden k-tiles of 128
BLK = 512            # samples per block
NSUB = BLK // 128    # 4 sub-tiles per block


def _build_program(n_shard: int):
    """Build the per-core Bass program for a shard of n_shard samples."""
    nblk = n_shard // BLK
    assert nblk * BLK == n_shard

    nc = bacc.Bacc(
        "TRN2",
        target_bir_lowering=False,
        debug=False,
        enable_asserts=False,
        num_devices=1,
    )

    x = nc.dram_tensor("x", [n_shard, D], F32, kind="ExternalInput").ap()
    w1t = nc.dram_tensor("w1t", [128, E * H], F32R, kind="ExternalInput").ap()
    w2r = nc.dram_tensor("w2r", [128, NJ, E * O], F32R, kind="ExternalInput").ap()
    b1c = nc.dram_tensor("b1c", [128, NJ], F32, kind="ExternalInput").ap()
    rvecb = nc.dram_tensor("rvecb", [128, D], F32, kind="ExternalInput").ap()
    b2bc = nc.dram_tensor("b2bc", [128, NSUB, E * O], F32, kind="ExternalInput").ap()
    ident = nc.dram_tensor("ident", [128, 128], F32, kind="ExternalInput").ap()
    thrv = nc.dram_tensor("thrv", [128, 1], F32, kind="ExternalInput").ap()
    out = nc.dram_tensor("out", [n_shard, O], F32, kind="ExternalOutput").ap()

    with tile.TileContext(nc) as tc:
        _body(tc, nblk, x, w1t, w2r, b1c, rvecb, b2bc, ident, thrv, out)

    nc.compile()
    return nc


def _body(tc, nblk, x, w1t, w2r, b1c, rvecb, b2bc, ident, thrv, out):
    nc = tc.nc
    Relu = mybir.ActivationFunctionType.Relu
    Alu = mybir.AluOpType

    with (
        tc.tile_pool(name="consts", bufs=1) as cpool,
        tc.tile_pool(name="xn", bufs=3) as xn_pool,
        tc.tile_pool(name="xt", bufs=2) as xt_pool,
        tc.tile_pool(name="h", bufs=3) as h_pool,
        tc.tile_pool(name="small", bufs=3) as s_pool,
        tc.tile_pool(name="xtp", bufs=2, space="PSUM") as xtp_pool,
        tc.tile_pool(name="zp", bufs=4, space="PSUM") as zp_pool,
        tc.tile_pool(name="op", bufs=1, space="PSUM") as op_pool,
        tc.tile_pool(name="ot", bufs=1, space="PSUM") as ot_pool,
    ):
        # --- load constants once ---
        w1t_sb = cpool.tile([128, E * H], F32R)
        nc.sync.dma_start(w1t_sb[:], w1t)
        w2r_sb = cpool.tile([128, NJ, E * O], F32R)
        nc.sync.dma_start(w2r_sb[:], w2r)
        b1c_sb = cpool.tile([128, NJ], F32)
        nc.sync.dma_start(b1c_sb[:], b1c)
        rvecb_sb = cpool.tile([128, D], F32)
        nc.sync.dma_start(rvecb_sb[:], rvecb)
        b2bc_sb = cpool.tile([128, NSUB, E * O], F32)
        nc.sync.dma_start(b2bc_sb[:], b2bc)
        id_sb = cpool.tile([128, 128], F32)
        nc.sync.dma_start(id_sb[:], ident)
        thr_sb = cpool.tile([128, 1], F32)
        nc.sync.dma_start(thr_sb[:], thrv)

        xv = x.rearrange("(n s p) d -> n p s d", p=128, s=NSUB)
        ov = out.rearrange("(n s p) o -> n p s o", p=128, s=NSUB)

        for bi in range(nblk):
            # 1. load natural x tiles [128b, 4s, 128d]
            xn = xn_pool.tile([128, NSUB, D], F32)
            nc.sync.dma_start(xn[:], xv[bi])

            # 2. transpose -> xT [128d, (s p)b]
            xtp = xtp_pool.tile([128, NSUB, 128], F32)
            for s in range(NSUB):
                nc.tensor.transpose(xtp[:, s, :], xn[:, s, :], id_sb[:])
            xt_sb = xt_pool.tile([128, BLK], F32R)
            nc.scalar.copy(xt_sb[:], xtp.rearrange("p s b -> p (s b)"))

            # 6. routing dot: q[b] = x[b] . rvec  (DVE, fp32)
            q_sb = s_pool.tile([128, NSUB], F32, tag="q")
            for s in range(NSUB):
                scr = s_pool.tile([128, D], F32, tag="scr")
                nc.vector.scalar_tensor_tensor(
                    out=scr[:],
                    in0=xn[:, s, :],
                    scalar=0.0,
                    in1=rvecb_sb[:],
                    op0=Alu.bypass,
                    op1=Alu.mult,
                    accum_out=q_sb[:, s : s + 1],
                )

            # 3. layer 1: z_j = w1_j^T @ xT   (fp32r)
            h = h_pool.tile([128, NJ, BLK], F32R)
            for j in range(NJ):
                zp = zp_pool.tile([128, BLK], F32)
                nc.tensor.matmul(
                    zp[:],
                    lhsT=w1t_sb[:, j * 128 : (j + 1) * 128],
                    rhs=xt_sb[:],
                    start=True,
                    stop=True,
                )
                # 4. relu(z + b1) -> h (fp32r), alternate ACT/DVE
                if j % 2 == 0:
                    nc.scalar.activation(
                        h[:, j, :], zp[:], Relu, bias=b1c_sb[:, j : j + 1], scale=1.0
                    )
                else:
                    nc.vector.tensor_scalar(
                        out=h[:, j, :],
                        in0=zp[:],
                        scalar1=b1c_sb[:, j : j + 1],
                        scalar2=0.0,
                        op0=Alu.add,
                        op1=Alu.max,
                    )

            # 5. layer 2 streaming: out_all [4(2e+o), 512b]
            op_ps = op_pool.tile([4, BLK], F32)
            for j in range(NJ):
                nc.tensor.matmul(
                    op_ps[:],
                    lhsT=w2r_sb[:, j, :],
                    rhs=h[:, j, :],
                    start=(j == 0),
                    stop=(j == NJ - 1),
                )
            oall_sb = s_pool.tile([4, BLK], F32, tag="oall")
            nc.scalar.copy(oall_sb[:], op_ps[:])

            # 7. transpose out_all to [128b, 4] + add b2 (rank-1 accumulate)
            ot_ps = ot_pool.tile([128, NSUB, E * O], F32)
            for s in range(NSUB):
                nc.tensor.matmul(
                    ot_ps[:, s, :],
                    lhsT=oall_sb[:, s * 128 : (s + 1) * 128],
                    rhs=id_sb[0:4, 0:4],
                    is_transpose=True,
                    start=True,
                    stop=True,
                )

            # select expert by routing mask, write out
            osb = s_pool.tile([128, NSUB, E * O], F32, tag="osb")
            nc.vector.tensor_tensor(osb[:], ot_ps[:], b2bc_sb[:], Alu.add)
            m_sb = s_pool.tile([128, NSUB], F32, tag="m")
            # expert0 wins ties: q <= thr -> 1.0
            nc.vector.tensor_scalar(
                out=m_sb[:],
                in0=q_sb[:],
                scalar1=thr_sb[:],
                scalar2=None,
                op0=Alu.is_le,
            )
            m2 = s_pool.tile([128, NSUB, O], F32, tag="m2")
            nc.vector.tensor_copy(m2[:], m_sb[:].broadcast_to([128, NSUB, O]))
            d_sb = s_pool.tile([128, NSUB, O], F32, tag="d")
            nc.vector.tensor_tensor(
                d_sb[:], osb[:, :, 0:O], osb[:, :, O : 2 * O], Alu.subtract
            )
            nc.vector.tensor_tensor(d_sb[:], d_sb[:], m2[:], Alu.mult)
            osel = s_pool.tile([128, NSUB, O], F32, tag="osel")
            nc.vector.tensor_tensor(
                osel[:], d_sb[:], osb[:, :, O : 2 * O], Alu.add
            )
            nc.sync.dma_start(ov[bi], osel[:])


def _pack_consts(w1, b1, w2, b2, prototypes):
    w1 = np.asarray(w1, np.float32)
    b1 = np.asarray(b1, np.float32)
    w2 = np.asarray(w2, np.float32)
    b2 = np.asarray(b2, np.float32)
    p = np.asarray(prototypes, np.float64)

    w1t = np.ascontiguousarray(np.transpose(w1, (2, 0, 1)).reshape(D, E * H))
    w2r = np.zeros((128, NJ, E * O), np.float32)
    b1c = np.zeros((128, NJ), np.float32)
    for e in range(E):
        for k in range(H // 128):
            j = e * (H // 128) + k
            for o in range(O):
                w2r[:, j, 2 * e + o] = w2[e, o, k * 128 : (k + 1) * 128]
            b1c[:, j] = b1[e, k * 128 : (k + 1) * 128]
    rvec = (p[1] - p[0]).astype(np.float32)
    rvecb = np.tile(rvec[None, :], (128, 1))
    thr = np.float32((p[1] @ p[1] - p[0] @ p[0]) / 2.0)
    thrv = np.full((128, 1), thr, np.float32)
    b2r = np.zeros((E * O,), np.float32)
    for e in range(E):
        for o in range(O):
            b2r[2 * e + o] = b2[e, o]
    b2bc = np.tile(b2r[None, None, :], (128, NSUB, 1))
    ident = np.eye(128, dtype=np.float32)
    return dict(
        w1t=w1t, w2r=w2r, b1c=b1c, rvecb=rvecb, b2bc=b2bc,
        ident=ident, thrv=thrv,
    )


_PROG_CACHE = {}


def _get_program(n_shard):
    if n_shard not in _PROG_CACHE:
        _PROG_CACHE[n_shard] = _build_program(n_shard)
    return _PROG_CACHE[n_shard]


def kernel(x, w1, b1, w2, b2, prototypes, _trace=False):
    x = np.ascontiguousarray(np.asarray(x, np.float32))
    btot = x.shape[0]
    n_shard = btot // N_CORES
    nc = _get_program(n_shard)
    consts = _pack_consts(w1, b1, w2, b2, prototypes)

    in_maps = []
    for c in range(N_CORES):
        m = dict(consts)
        m["x"] = x[c * n_shard : (c + 1) * n_shard]
        in_maps.append(m)

    res = run_bass_kernel_spmd(
        nc, in_maps, core_ids=list(range(N_CORES)), trace=_trace
    )
    outs = [res.results[c]["out"] for c in range(N_CORES)]
    full = np.concatenate(outs, axis=0)
    if _trace:
        return full, res
    return full
